# revision 4
# baseline (speedup 1.0000x reference)
"""Trainium2 Bass kernel for nn_BasicTransformerBlock_18657337934637.

Sparse-attention transformer block:
  q/k/v = hidden @ W* + b*        (2304 -> 2304, 24 heads x 96)
  RoPE3D on q, k
  sparse-1d grouping (SPARSE_N=4): token t -> group t%4, 1024 tokens/group
  softmax attention within each (group, head)
  out = attn @ wo + bo

Distribution over 8 NeuronCores:
  Launch 1 (head-parallel): core c computes heads 3c..3c+2 end-to-end through
    attention.  Host pre-transposes hidden to hT [2304, 4096] in grouped token
    order, so groups are contiguous 1024-token spans and the QKV matmuls need
    no on-device transpose of the activations.  Per (group, head): scores are
    computed transposed [k, q] so softmax-exp sums and the P@V contraction both
    run with k on the partition axis; an all-ones column appended to v yields
    the softmax denominator for free in the same matmul; exp skips the max
    subtraction (scores are O(5), fp32 exp is safe).  Output: normalized
    attn^T [3, 96, 4096] per core.
  Host: gather heads -> attnT [2304, 4096], undo token permutation.
  Launch 2 (token x outdim parallel): core (i, j) computes
    out[i*1024:(i+1)*1024, j*1152:(j+1)*1152] = attnT_i.T @ wo_j + bo_j.

Matmuls run as float32r (full fp32 storage; TensorE single-pass mode, 1 row/cyc
for moving dim >= 256).  Set KERNEL_MM_DT=f32 to fall back to exact-fp32
two-pass matmuls.
"""
import os
import numpy as np

HEADS = 24
HD = 96
SPN = 4
S = 4096
DIM = 2304
KC = DIM // 128            # 18 contraction chunks
HPC = 3                    # heads per core
CW = HPC * HD              # 288 columns per core
G = S // SPN               # 1024 tokens per group
NTB = S // 128             # 32 token tiles
SCALE = 1.0 / float(np.sqrt(HD))

_CACHE = {}
LAST_RESULTS = []          # test harness introspection


def _mm_dt():
    import concourse.mybir as mybir
    return (mybir.dt.float32 if os.environ.get("KERNEL_MM_DT") == "f32"
            else mybir.dt.float32r)


def _build_launch1():
    import concourse.mybir as mybir
    import concourse.tile as tile
    from concourse import bacc
    from concourse.masks import make_identity

    f32 = mybir.dt.float32
    mm = _mm_dt()
    nc = bacc.Bacc("TRN2", target_bir_lowering=False, debug=False)

    hT_d = nc.dram_tensor("hT", [DIM, S], f32, kind="ExternalInput").ap()
    w_d = {n: nc.dram_tensor(n, [DIM, CW], f32, kind="ExternalInput").ap()
           for n in ("wq", "wk", "wv")}
    b_d = {n: nc.dram_tensor(n, [1, CW], f32, kind="ExternalInput").ap()
           for n in ("bq", "bk", "bv")}
    A3_d = nc.dram_tensor("A3", [S, CW], f32, kind="ExternalInput").ap()
    B3_d = nc.dram_tensor("B3", [S, CW], f32, kind="ExternalInput").ap()
    outN_d = nc.dram_tensor("outN", [HPC, HD + 1, S], f32, kind="ExternalOutput").ap()

    with tile.TileContext(nc) as tc:
        with (
            tc.tile_pool(name="singles", bufs=1) as singles,
            tc.tile_pool(name="hp", bufs=3) as hp,
            tc.tile_pool(name="rp", bufs=3) as rp,
            tc.tile_pool(name="qkp", bufs=3) as qkp,
            tc.tile_pool(name="vp", bufs=16) as vp,
            tc.tile_pool(name="qtp", bufs=6) as qtp,
            tc.tile_pool(name="ktp", bufs=6) as ktp,
            tc.tile_pool(name="ep", bufs=3) as ep,
            tc.tile_pool(name="op", bufs=3) as op,
            tc.tile_pool(name="ppq", bufs=2, space="PSUM") as ppq,
            tc.tile_pool(name="ppt", bufs=2, space="PSUM") as ppt,
            tc.tile_pool(name="pps", bufs=2, space="PSUM") as pps,
            tc.tile_pool(name="ppv", bufs=2, space="PSUM") as ppv,
        ):
            ident = singles.tile([128, 128], f32, tag="ident", name="ident")
            make_identity(nc, ident)
            # resident weights: [128, 18*288] with chunk-major free layout
            w_sb, b_sb = {}, {}
            for n in ("wq", "wk", "wv"):
                t = singles.tile([128, KC * CW], f32, tag=f"{n}_sb",
                                 name=f"{n}_sb")
                nc.sync.dma_start(
                    t.rearrange("p (k c) -> p k c", k=KC),
                    w_d[n].rearrange("(k p) c -> p k c", p=128))
                w_sb[n] = t.rearrange("p (k c) -> p k c", k=KC)
            for n in ("bq", "bk", "bv"):
                t = singles.tile([128, CW], f32, tag=f"{n}_sb", name=f"{n}_sb")
                nc.gpsimd.dma_start(out=t, in_=b_d[n].to_broadcast([128, CW]))
                b_sb[n] = t

            qT, kT, vt = {}, {}, {}

            def attention(g):
                for h in range(HPC):
                    for qh in range(2):
                        pv = ppv.tile([HD + 1, 512], f32, tag="pv",
                                      name=f"pv{g}_{h}_{qh}")
                        qs = qT[(g, h)][:, qh * 512:(qh + 1) * 512].bitcast(mm)
                        for kc in range(8):
                            st = pps.tile([128, 512], f32, tag="st",
                                          name=f"st{g}_{h}_{qh}_{kc}")
                            nc.tensor.matmul(
                                st,
                                kT[(g, h)][:, kc * 128:(kc + 1) * 128].bitcast(mm),
                                qs, start=True, stop=True)
                            ex = ep.tile([128, 512], f32, tag="ex",
                                         name=f"ex{g}_{h}_{qh}_{kc}")
                            nc.scalar.activation(
                                ex, st, mybir.ActivationFunctionType.Exp,
                                scale=SCALE)
                            nc.tensor.matmul(
                                pv,
                                vt[(g, kc)][:, h * 97:(h + 1) * 97].bitcast(mm),
                                ex.bitcast(mm),
                                start=(kc == 0), stop=(kc == 7))
                        ot = op.tile([HD + 1, 512], f32, tag="ot",
                                     name=f"ot{g}_{h}_{qh}")
                        nc.scalar.copy(ot, pv)
                        nc.sync.dma_start(
                            outN_d[h, :, g * G + qh * 512:g * G + (qh + 1) * 512],
                            ot)

            for tb in range(NTB):
                g, col = tb // 8, (tb % 8) * 128
                if tb % 8 == 0:
                    for h in range(HPC):
                        qT[(g, h)] = qtp.tile([HD, G], f32, tag="qT",
                                              name=f"qT{g}_{h}")
                        kT[(g, h)] = ktp.tile([HD, G], f32, tag="kT",
                                              name=f"kT{g}_{h}")
                ht = hp.tile([128, KC * 128], f32, tag="ht", name=f"ht{tb}")
                nc.sync.dma_start(
                    ht.rearrange("p (k t) -> p k t", k=KC),
                    hT_d[:, tb * 128:(tb + 1) * 128]
                    .rearrange("(k p) t -> p k t", p=128))
                htv = ht.rearrange("p (k t) -> p k t", k=KC)
                a3 = rp.tile([128, CW], f32, tag="a3", name=f"a3_{tb}")
                nc.sync.dma_start(a3, A3_d[tb * 128:(tb + 1) * 128, :])
                b3 = rp.tile([128, CW], f32, tag="b3", name=f"b3_{tb}")
                nc.sync.dma_start(b3, B3_d[tb * 128:(tb + 1) * 128, :])

                for n, dest in (("wq", "q"), ("wk", "k"), ("wv", "v")):
                    ps = ppq.tile([128, CW], f32, tag="ps", name=f"ps_{n}{tb}")
                    for kc in range(KC):
                        nc.tensor.matmul(
                            ps, htv[:, kc, :].bitcast(mm),
                            w_sb[n][:, kc, :].bitcast(mm),
                            start=(kc == 0), stop=(kc == KC - 1))
                    bias = b_sb["b" + n[1]]
                    if dest == "v":
                        v_t = vp.tile([128, HPC * (HD + 1)], f32, tag="v",
                                      name=f"v{tb}")
                        for h in range(HPC):
                            nc.vector.tensor_tensor(
                                v_t[:, h * 97:h * 97 + 96],
                                ps[:, h * 96:(h + 1) * 96],
                                bias[:, h * 96:(h + 1) * 96],
                                mybir.AluOpType.add)
                        nc.vector.memset(
                            v_t.rearrange("p (h c) -> p h c", h=HPC)[:, :, 96:97],
                            1.0)
                        vt[(g, tb % 8)] = v_t
                    else:
                        q_sb = qkp.tile([128, CW], f32, tag=f"{dest}sb",
                                        name=f"{dest}sb{tb}")
                        nc.vector.tensor_tensor(q_sb, ps, bias,
                                                mybir.AluOpType.add)
                        shf = qkp.tile([128, CW], f32, tag="shf",
                                       name=f"shf_{dest}{tb}")
                        qv = q_sb.rearrange("p (h c u f) -> p h c u f",
                                            h=3, c=3, u=2)
                        sv = shf.rearrange("p (h c u f) -> p h c u f",
                                           h=3, c=3, u=2)
                        nc.vector.tensor_copy(sv[:, :, :, 0:1, :],
                                              qv[:, :, :, 1:2, :])
                        nc.vector.tensor_copy(sv[:, :, :, 1:2, :],
                                              qv[:, :, :, 0:1, :])
                        nc.vector.tensor_tensor(shf, shf, b3,
                                                mybir.AluOpType.mult)
                        nc.vector.tensor_tensor(q_sb, q_sb, a3,
                                                mybir.AluOpType.mult)
                        nc.vector.tensor_tensor(q_sb, q_sb, shf,
                                                mybir.AluOpType.add)
                        dst = qT if dest == "q" else kT
                        for h in range(HPC):
                            pt = ppt.tile([HD, 128], f32, tag="pt",
                                          name=f"pt_{dest}{tb}_{h}")
                            nc.tensor.transpose(
                                pt, q_sb[:, h * 96:(h + 1) * 96], ident)
                            nc.scalar.copy(
                                dst[(g, h)][:, col:col + 128], pt)
                if tb % 8 == 7:
                    attention(g)
    nc.compile()
    return nc


def _build_launch2():
    import concourse.mybir as mybir
    import concourse.tile as tile
    from concourse import bacc

    f32 = mybir.dt.float32
    mm = _mm_dt()
    TOK = 1024           # tokens per core
    NW = 1152            # outdims per core
    NP = 3               # psum pieces of 384
    nc = bacc.Bacc("TRN2", target_bir_lowering=False, debug=False)

    at_d = nc.dram_tensor("attnT", [DIM, TOK], f32, kind="ExternalInput").ap()
    wo_d = nc.dram_tensor("woj", [DIM, NW], f32, kind="ExternalInput").ap()
    bo_d = nc.dram_tensor("boj", [1, NW], f32, kind="ExternalInput").ap()
    out_d = nc.dram_tensor("out", [TOK, NW], f32, kind="ExternalOutput").ap()

    with tile.TileContext(nc) as tc:
        ats, wos = [], []
        with (
            tc.tile_pool(name="singles2", bufs=1) as singles,
            tc.tile_pool(name="atp", bufs=KC) as atp,
            tc.tile_pool(name="wop", bufs=KC) as wop,
            tc.tile_pool(name="outp", bufs=3) as outp,
            tc.tile_pool(name="psp", bufs=4, space="PSUM") as psp,
        ):
            bo_sb = singles.tile([128, NW], f32, tag="bo_sb", name="bo_sb")
            nc.gpsimd.dma_start(out=bo_sb, in_=bo_d.to_broadcast([128, NW]))
            for kc in range(KC):
                a = atp.tile([128, TOK], f32, tag="at", name=f"at{kc}")
                nc.sync.dma_start(a, at_d[kc * 128:(kc + 1) * 128, :])
                ats.append(a)
                w = wop.tile([128, NW], f32, tag="wo", name=f"wo{kc}")
                nc.sync.dma_start(w, wo_d[kc * 128:(kc + 1) * 128, :])
                wos.append(w)
            for m in range(TOK // 128):
                ot = outp.tile([128, NW], f32, tag="ot", name=f"ot{m}")
                for np_ in range(NP):
                    ps = psp.tile([128, 384], f32, tag="ps", name=f"ps{m}_{np_}")
                    for kc in range(KC):
                        nc.tensor.matmul(
                            ps, ats[kc][:, m * 128:(m + 1) * 128].bitcast(mm),
                            wos[kc][:, np_ * 384:(np_ + 1) * 384].bitcast(mm),
                            start=(kc == 0), stop=(kc == KC - 1))
                    nc.vector.tensor_tensor(
                        ot[:, np_ * 384:(np_ + 1) * 384], ps,
                        bo_sb[:, np_ * 384:(np_ + 1) * 384],
                        mybir.AluOpType.add)
                nc.sync.dma_start(out_d[m * 128:(m + 1) * 128, :], ot)
    nc.compile()
    return nc


def _get(name, builder):
    if name not in _CACHE:
        _CACHE[name] = builder()
    return _CACHE[name]


def _rope_tables(frame, height, width):
    t = np.repeat(np.arange(frame), height * width)
    y = np.tile(np.repeat(np.arange(height), width), frame)
    x = np.tile(np.arange(width), frame * height)
    D = HD // 3
    A = np.empty((S, HD), np.float32)
    B = np.empty((S, HD), np.float32)
    for i, pos in enumerate((t, y, x)):
        inv = 1.0 / (10000.0 ** (np.arange(0, D, 2, dtype=np.float32) / D))
        f = pos[:, None].astype(np.float32) * inv[None, :]
        A[:, i * D:i * D + 16] = np.cos(f)
        A[:, i * D + 16:(i + 1) * D] = np.cos(f)
        B[:, i * D:i * D + 16] = -np.sin(f)
        B[:, i * D + 16:(i + 1) * D] = np.sin(f)
    return A, B


def kernel(hidden_states, wq, bq, wk, bk, wv, bv, wo, bo, frame, height, width):
    from concourse import bass_utils

    f, hh, ww = int(frame), int(height), int(width)
    hs = np.asarray(hidden_states, dtype=np.float32)
    assert hs.shape == (1, S, DIM) and f * hh * ww == S
    wq, wk, wv, wo = (np.asarray(a, np.float32) for a in (wq, wk, wv, wo))
    bq, bk, bv, bo = (np.asarray(a, np.float32) for a in (bq, bk, bv, bo))

    perm = np.concatenate([np.arange(k, S, SPN) for k in range(SPN)])
    A, B = _rope_tables(f, hh, ww)
    A3 = np.ascontiguousarray(np.tile(A[perm], (1, HPC)))
    B3 = np.ascontiguousarray(np.tile(B[perm], (1, HPC)))
    hT = np.ascontiguousarray(hs[0].T[:, perm])

    nc1 = _get("l1", _build_launch1)
    in1 = []
    for c in range(8):
        sl = slice(c * CW, (c + 1) * CW)
        in1.append({
            "hT": hT,
            "wq": np.ascontiguousarray(wq[:, sl]),
            "wk": np.ascontiguousarray(wk[:, sl]),
            "wv": np.ascontiguousarray(wv[:, sl]),
            "bq": np.ascontiguousarray(bq[sl]).reshape(1, CW),
            "bk": np.ascontiguousarray(bk[sl]).reshape(1, CW),
            "bv": np.ascontiguousarray(bv[sl]).reshape(1, CW),
            "A3": A3, "B3": B3,
        })
    res1 = bass_utils.run_bass_kernel_spmd(nc1, in1, core_ids=list(range(8)))
    LAST_RESULTS.append(res1)

    outN = np.concatenate([res1.results[c]["outN"] for c in range(8)], 0)
    attnT_g = (outN[:, :HD, :] / outN[:, HD:HD + 1, :]).reshape(DIM, S)
    attnT = np.empty_like(attnT_g)
    attnT[:, perm] = attnT_g

    nc2 = _get("l2", _build_launch2)
    in2 = []
    for c in range(8):
        i, j = divmod(c, 2)
        in2.append({
            "attnT": np.ascontiguousarray(attnT[:, i * 1024:(i + 1) * 1024]),
            "woj": np.ascontiguousarray(wo[:, j * 1152:(j + 1) * 1152]),
            "boj": np.ascontiguousarray(bo[j * 1152:(j + 1) * 1152]).reshape(1, 1152),
        })
    res2 = bass_utils.run_bass_kernel_spmd(nc2, in2, core_ids=list(range(8)))
    LAST_RESULTS.append(res2)

    out = np.empty((S, DIM), np.float32)
    for c in range(8):
        i, j = divmod(c, 2)
        out[i * 1024:(i + 1) * 1024, j * 1152:(j + 1) * 1152] = \
            res2.results[c]["out"]
    return out[None]


# revision 6
# speedup vs baseline: 2.6009x; 2.6009x over previous
"""Trainium2 Bass kernel for nn_BasicTransformerBlock_18657337934637.

Sparse-attention transformer block:
  q/k/v = hidden @ W* + b*        (2304 -> 2304, 24 heads x 96)
  RoPE3D on q, k
  sparse-1d grouping (SPARSE_N=4): token t -> group t%4, 1024 tokens/group
  softmax attention within each (group, head)
  out = attn @ wo + bo

Distribution over 8 NeuronCores:
  Launch 1 (head-parallel): core c computes heads 3c..3c+2 end-to-end through
    attention.  Host pre-transposes hidden to hT [2304, 4096] in grouped token
    order, so groups are contiguous 1024-token spans and the QKV matmuls need
    no on-device transpose of the activations.  Per (group, head): scores are
    computed transposed [k, q] so softmax-exp sums and the P@V contraction both
    run with k on the partition axis; an all-ones column appended to v yields
    the softmax denominator for free in the same matmul; exp skips the max
    subtraction (scores are O(5), fp32 exp is safe).  Output: normalized
    attn^T [3, 96, 4096] per core.
  Host: gather heads -> attnT [2304, 4096], undo token permutation.
  Launch 2 (token x outdim parallel): core (i, j) computes
    out[i*1024:(i+1)*1024, j*1152:(j+1)*1152] = attnT_i.T @ wo_j + bo_j.

Matmuls run as float32r (full fp32 storage; TensorE single-pass mode, 1 row/cyc
for moving dim >= 256).  Set KERNEL_MM_DT=f32 to fall back to exact-fp32
two-pass matmuls.
"""
import os
import numpy as np

HEADS = 24
HD = 96
SPN = 4
S = 4096
DIM = 2304
KC = DIM // 128            # 18 contraction chunks
HPC = 3                    # heads per core
CW = HPC * HD              # 288 columns per core
G = S // SPN               # 1024 tokens per group
NTB = S // 128             # 32 token tiles
SCALE = 1.0 / float(np.sqrt(HD))

_CACHE = {}
LAST_RESULTS = []          # test harness introspection


def _mm_dt():
    import concourse.mybir as mybir
    return (mybir.dt.float32 if os.environ.get("KERNEL_MM_DT") == "f32"
            else mybir.dt.float32r)


def _build_launch1():
    import concourse.mybir as mybir
    import concourse.tile as tile
    from concourse import bacc
    from concourse.masks import make_identity

    f32 = mybir.dt.float32
    mm = _mm_dt()
    nc = bacc.Bacc("TRN2", target_bir_lowering=False, debug=False)

    hT_d = nc.dram_tensor("hT", [DIM, S], mm, kind="ExternalInput").ap()
    w_d = {n: nc.dram_tensor(n, [DIM, CW], mm, kind="ExternalInput").ap()
           for n in ("wq", "wk", "wv")}
    b_d = {n: nc.dram_tensor(n, [1, CW], f32, kind="ExternalInput").ap()
           for n in ("bq", "bk", "bv")}
    A3_d = nc.dram_tensor("A3", [S, CW], f32, kind="ExternalInput").ap()
    B3_d = nc.dram_tensor("B3", [S, CW], f32, kind="ExternalInput").ap()
    outN_d = nc.dram_tensor("outN", [HPC, HD + 1, S], f32, kind="ExternalOutput").ap()

    with tile.TileContext(nc) as tc:
        with (
            tc.tile_pool(name="singles", bufs=1) as singles,
            tc.tile_pool(name="hp", bufs=3) as hp,
            tc.tile_pool(name="rp", bufs=3) as rp,
            tc.tile_pool(name="qkp", bufs=3) as qkp,
            tc.tile_pool(name="vp", bufs=16) as vp,
            tc.tile_pool(name="qtp", bufs=6) as qtp,
            tc.tile_pool(name="ktp", bufs=6) as ktp,
            tc.tile_pool(name="ep", bufs=3) as ep,
            tc.tile_pool(name="op", bufs=3) as op,
            tc.tile_pool(name="ppq", bufs=2, space="PSUM") as ppq,
            tc.tile_pool(name="ppt", bufs=2, space="PSUM") as ppt,
            tc.tile_pool(name="pps", bufs=2, space="PSUM") as pps,
            tc.tile_pool(name="ppv", bufs=2, space="PSUM") as ppv,
        ):
            ident = singles.tile([128, 128], f32, tag="ident", name="ident")
            make_identity(nc, ident)
            # resident weights: [128, 18*288] with chunk-major free layout
            w_sb, b_sb = {}, {}
            for n in ("wq", "wk", "wv"):
                t = singles.tile([128, KC * CW], mm, tag=f"{n}_sb",
                                 name=f"{n}_sb")
                nc.sync.dma_start(
                    t.rearrange("p (k c) -> p k c", k=KC),
                    w_d[n].rearrange("(k p) c -> p k c", p=128))
                w_sb[n] = t.rearrange("p (k c) -> p k c", k=KC)
            for n in ("bq", "bk", "bv"):
                t = singles.tile([128, CW], f32, tag=f"{n}_sb", name=f"{n}_sb")
                nc.gpsimd.dma_start(out=t, in_=b_d[n].to_broadcast([128, CW]))
                b_sb[n] = t
            ones3 = singles.tile([128, HPC], f32, tag="ones3", name="ones3")
            nc.vector.memset(ones3, 1.0)

            qT, kT, vt = {}, {}, {}

            def attention(g):
                for h in range(HPC):
                    for qh in range(2):
                        pv = ppv.tile([HD + 1, 512], f32, tag="pv",
                                      name=f"pv{g}_{h}_{qh}")
                        qs = qT[(g, h)][:, qh * 512:(qh + 1) * 512]
                        for kc in range(8):
                            st = pps.tile([128, 512], f32, tag="st",
                                          name=f"st{g}_{h}_{qh}_{kc}")
                            nc.tensor.matmul(
                                st,
                                kT[(g, h)][:, kc * 128:(kc + 1) * 128],
                                qs, start=True, stop=True)
                            ex = ep.tile([128, 512], mm, tag="ex",
                                         name=f"ex{g}_{h}_{qh}_{kc}")
                            nc.scalar.activation(
                                ex, st, mybir.ActivationFunctionType.Exp,
                                scale=SCALE)
                            nc.tensor.matmul(
                                pv,
                                vt[(g, kc)][:, h * 97:(h + 1) * 97],
                                ex,
                                start=(kc == 0), stop=(kc == 7))
                        ot = op.tile([HD + 1, 512], f32, tag="ot",
                                     name=f"ot{g}_{h}_{qh}")
                        nc.scalar.copy(ot, pv)
                        nc.sync.dma_start(
                            outN_d[h, :, g * G + qh * 512:g * G + (qh + 1) * 512],
                            ot)

            for tb in range(NTB):
                g, col = tb // 8, (tb % 8) * 128
                if tb % 8 == 0:
                    for h in range(HPC):
                        qT[(g, h)] = qtp.tile([HD, G], mm, tag="qT",
                                              name=f"qT{g}_{h}")
                        kT[(g, h)] = ktp.tile([HD, G], mm, tag="kT",
                                              name=f"kT{g}_{h}")
                ht = hp.tile([128, KC * 128], mm, tag="ht", name=f"ht{tb}")
                nc.sync.dma_start(
                    ht.rearrange("p (k t) -> p k t", k=KC),
                    hT_d[:, tb * 128:(tb + 1) * 128]
                    .rearrange("(k p) t -> p k t", p=128))
                htv = ht.rearrange("p (k t) -> p k t", k=KC)
                a3 = rp.tile([128, CW], f32, tag="a3", name=f"a3_{tb}")
                nc.sync.dma_start(a3, A3_d[tb * 128:(tb + 1) * 128, :])
                b3 = rp.tile([128, CW], f32, tag="b3", name=f"b3_{tb}")
                nc.sync.dma_start(b3, B3_d[tb * 128:(tb + 1) * 128, :])

                for n, dest in (("wq", "q"), ("wk", "k"), ("wv", "v")):
                    ps = ppq.tile([128, CW], f32, tag="ps", name=f"ps_{n}{tb}")
                    for kc in range(KC):
                        nc.tensor.matmul(
                            ps, htv[:, kc, :], w_sb[n][:, kc, :],
                            start=(kc == 0), stop=(kc == KC - 1))
                    bias = b_sb["b" + n[1]]
                    if dest == "v":
                        v_t = vp.tile([128, HPC * (HD + 1)], mm, tag="v",
                                      name=f"v{tb}")
                        for h in range(HPC):
                            nc.vector.tensor_tensor(
                                v_t[:, h * 97:h * 97 + 96],
                                ps[:, h * 96:(h + 1) * 96],
                                bias[:, h * 96:(h + 1) * 96],
                                mybir.AluOpType.add)
                        nc.vector.tensor_copy(
                            v_t.rearrange("p (h c) -> p h c", h=HPC)[:, :, 96:97],
                            ones3.rearrange("p (h c) -> p h c", h=HPC))
                        vt[(g, tb % 8)] = v_t
                    else:
                        q_sb = qkp.tile([128, CW], f32, tag=f"{dest}sb",
                                        name=f"{dest}sb{tb}")
                        nc.vector.tensor_tensor(q_sb, ps, bias,
                                                mybir.AluOpType.add)
                        shf = qkp.tile([128, CW], f32, tag="shf",
                                       name=f"shf_{dest}{tb}")
                        qv = q_sb.rearrange("p (h c u f) -> p h c u f",
                                            h=3, c=3, u=2)
                        sv = shf.rearrange("p (h c u f) -> p h c u f",
                                           h=3, c=3, u=2)
                        nc.vector.tensor_copy(sv[:, :, :, 0:1, :],
                                              qv[:, :, :, 1:2, :])
                        nc.vector.tensor_copy(sv[:, :, :, 1:2, :],
                                              qv[:, :, :, 0:1, :])
                        nc.vector.tensor_tensor(shf, shf, b3,
                                                mybir.AluOpType.mult)
                        nc.vector.tensor_tensor(q_sb, q_sb, a3,
                                                mybir.AluOpType.mult)
                        nc.vector.tensor_tensor(q_sb, q_sb, shf,
                                                mybir.AluOpType.add)
                        dst = qT if dest == "q" else kT
                        for h in range(HPC):
                            pt = ppt.tile([HD, 128], f32, tag="pt",
                                          name=f"pt_{dest}{tb}_{h}")
                            nc.tensor.transpose(
                                pt, q_sb[:, h * 96:(h + 1) * 96], ident)
                            nc.scalar.copy(
                                dst[(g, h)][:, col:col + 128], pt)
                if tb % 8 == 7:
                    attention(g)
    nc.compile()
    return nc


def _build_launch2():
    import concourse.mybir as mybir
    import concourse.tile as tile
    from concourse import bacc

    f32 = mybir.dt.float32
    mm = _mm_dt()
    TOK = 1024           # tokens per core
    NW = 1152            # outdims per core
    NP = 3               # psum pieces of 384
    nc = bacc.Bacc("TRN2", target_bir_lowering=False, debug=False)

    at_d = nc.dram_tensor("attnT", [DIM, TOK], mm, kind="ExternalInput").ap()
    wo_d = nc.dram_tensor("woj", [DIM, NW], mm, kind="ExternalInput").ap()
    bo_d = nc.dram_tensor("boj", [1, NW], f32, kind="ExternalInput").ap()
    out_d = nc.dram_tensor("out", [TOK, NW], f32, kind="ExternalOutput").ap()

    with tile.TileContext(nc) as tc:
        ats, wos = [], []
        with (
            tc.tile_pool(name="singles2", bufs=1) as singles,
            tc.tile_pool(name="atp", bufs=KC) as atp,
            tc.tile_pool(name="wop", bufs=KC) as wop,
            tc.tile_pool(name="outp", bufs=3) as outp,
            tc.tile_pool(name="psp", bufs=4, space="PSUM") as psp,
        ):
            bo_sb = singles.tile([128, NW], f32, tag="bo_sb", name="bo_sb")
            nc.gpsimd.dma_start(out=bo_sb, in_=bo_d.to_broadcast([128, NW]))
            for kc in range(KC):
                a = atp.tile([128, TOK], mm, tag="at", name=f"at{kc}")
                nc.sync.dma_start(a, at_d[kc * 128:(kc + 1) * 128, :])
                ats.append(a)
                w = wop.tile([128, NW], mm, tag="wo", name=f"wo{kc}")
                nc.sync.dma_start(w, wo_d[kc * 128:(kc + 1) * 128, :])
                wos.append(w)
            for m in range(TOK // 128):
                ot = outp.tile([128, NW], f32, tag="ot", name=f"ot{m}")
                for np_ in range(NP):
                    ps = psp.tile([128, 384], f32, tag="ps", name=f"ps{m}_{np_}")
                    for kc in range(KC):
                        nc.tensor.matmul(
                            ps, ats[kc][:, m * 128:(m + 1) * 128],
                            wos[kc][:, np_ * 384:(np_ + 1) * 384],
                            start=(kc == 0), stop=(kc == KC - 1))
                    nc.vector.tensor_tensor(
                        ot[:, np_ * 384:(np_ + 1) * 384], ps,
                        bo_sb[:, np_ * 384:(np_ + 1) * 384],
                        mybir.AluOpType.add)
                nc.sync.dma_start(out_d[m * 128:(m + 1) * 128, :], ot)
    nc.compile()
    return nc


def _get(name, builder):
    if name not in _CACHE:
        _CACHE[name] = builder()
    return _CACHE[name]


def _rope_tables(frame, height, width):
    t = np.repeat(np.arange(frame), height * width)
    y = np.tile(np.repeat(np.arange(height), width), frame)
    x = np.tile(np.arange(width), frame * height)
    D = HD // 3
    A = np.empty((S, HD), np.float32)
    B = np.empty((S, HD), np.float32)
    for i, pos in enumerate((t, y, x)):
        inv = 1.0 / (10000.0 ** (np.arange(0, D, 2, dtype=np.float32) / D))
        f = pos[:, None].astype(np.float32) * inv[None, :]
        A[:, i * D:i * D + 16] = np.cos(f)
        A[:, i * D + 16:(i + 1) * D] = np.cos(f)
        B[:, i * D:i * D + 16] = -np.sin(f)
        B[:, i * D + 16:(i + 1) * D] = np.sin(f)
    return A, B


def kernel(hidden_states, wq, bq, wk, bk, wv, bv, wo, bo, frame, height, width):
    from concourse import bass_utils

    f, hh, ww = int(frame), int(height), int(width)
    hs = np.asarray(hidden_states, dtype=np.float32)
    assert hs.shape == (1, S, DIM) and f * hh * ww == S
    wq, wk, wv, wo = (np.asarray(a, np.float32) for a in (wq, wk, wv, wo))
    bq, bk, bv, bo = (np.asarray(a, np.float32) for a in (bq, bk, bv, bo))

    perm = np.concatenate([np.arange(k, S, SPN) for k in range(SPN)])
    A, B = _rope_tables(f, hh, ww)
    A3 = np.ascontiguousarray(np.tile(A[perm], (1, HPC)))
    B3 = np.ascontiguousarray(np.tile(B[perm], (1, HPC)))
    hT = np.ascontiguousarray(hs[0].T[:, perm])

    nc1 = _get("l1", _build_launch1)
    in1 = []
    for c in range(8):
        sl = slice(c * CW, (c + 1) * CW)
        in1.append({
            "hT": hT,
            "wq": np.ascontiguousarray(wq[:, sl]),
            "wk": np.ascontiguousarray(wk[:, sl]),
            "wv": np.ascontiguousarray(wv[:, sl]),
            "bq": np.ascontiguousarray(bq[sl]).reshape(1, CW),
            "bk": np.ascontiguousarray(bk[sl]).reshape(1, CW),
            "bv": np.ascontiguousarray(bv[sl]).reshape(1, CW),
            "A3": A3, "B3": B3,
        })
    res1 = bass_utils.run_bass_kernel_spmd(nc1, in1, core_ids=list(range(8)))
    LAST_RESULTS.append(res1)

    outN = np.concatenate([res1.results[c]["outN"] for c in range(8)], 0)
    attnT_g = (outN[:, :HD, :] / outN[:, HD:HD + 1, :]).reshape(DIM, S)
    attnT = np.empty_like(attnT_g)
    attnT[:, perm] = attnT_g

    nc2 = _get("l2", _build_launch2)
    in2 = []
    for c in range(8):
        i, j = divmod(c, 2)
        in2.append({
            "attnT": np.ascontiguousarray(attnT[:, i * 1024:(i + 1) * 1024]),
            "woj": np.ascontiguousarray(wo[:, j * 1152:(j + 1) * 1152]),
            "boj": np.ascontiguousarray(bo[j * 1152:(j + 1) * 1152]).reshape(1, 1152),
        })
    res2 = bass_utils.run_bass_kernel_spmd(nc2, in2, core_ids=list(range(8)))
    LAST_RESULTS.append(res2)

    out = np.empty((S, DIM), np.float32)
    for c in range(8):
        i, j = divmod(c, 2)
        out[i * 1024:(i + 1) * 1024, j * 1152:(j + 1) * 1152] = \
            res2.results[c]["out"]
    return out[None]


# revision 8
# speedup vs baseline: 2.7683x; 1.0644x over previous
"""Trainium2 Bass kernel for nn_BasicTransformerBlock_18657337934637.

Sparse-attention transformer block:
  q/k/v = hidden @ W* + b*        (2304 -> 2304, 24 heads x 96)
  RoPE3D on q, k
  sparse-1d grouping (SPARSE_N=4): token t -> group t%4, 1024 tokens/group
  softmax attention within each (group, head)
  out = attn @ wo + bo

Distribution over 8 NeuronCores:
  Launch 1 (head-parallel): core c computes heads 3c..3c+2 end-to-end through
    attention.  Host pre-transposes hidden to hT [2304, 4096] in grouped token
    order, so groups are contiguous 1024-token spans and the QKV matmuls need
    no on-device transpose of the activations.  Per (group, head): scores are
    computed transposed [k, q] so softmax-exp sums and the P@V contraction both
    run with k on the partition axis; an all-ones column appended to v yields
    the softmax denominator for free in the same matmul; exp skips the max
    subtraction (scores are O(5), fp32 exp is safe).  Output: un-normalized
    attn^T + denominator row, [3, 97, 4096] per core; the host divides.
  Host: gather heads -> attnT [2304, 4096], undo token permutation.
  Launch 2 (token x outdim parallel): core (i, j) computes
    out[i*1024:(i+1)*1024, j*1152:(j+1)*1152]^T = wo_j^T @ attnT_i
    (output kept transposed so the weight stays stationary on the PE).

Matmuls run as float32r (full fp32 storage; TensorE single-pass mode, 1 row/cyc
for moving dim >= 256).  Set KERNEL_MM_DT=f32 to fall back to exact-fp32
two-pass matmuls.
"""
import os
import numpy as np

HEADS = 24
HD = 96
SPN = 4
S = 4096
DIM = 2304
KC = DIM // 128            # 18 contraction chunks
HPC = 3                    # heads per core
CW = HPC * HD              # 288 columns per core
G = S // SPN               # 1024 tokens per group
TB = 256                   # hT dma block (tokens)
NB = S // TB               # 16 blocks
SCALE = 1.0 / float(np.sqrt(HD))

_CACHE = {}
LAST_RESULTS = []          # test harness introspection


def _mm_dt():
    import concourse.mybir as mybir
    return (mybir.dt.float32 if os.environ.get("KERNEL_MM_DT") == "f32"
            else mybir.dt.float32r)


def _build_launch1():
    import concourse.mybir as mybir
    import concourse.tile as tile
    from concourse import bacc
    from concourse.masks import make_identity

    f32 = mybir.dt.float32
    mm = _mm_dt()
    Exp = mybir.ActivationFunctionType.Exp
    MUL = mybir.AluOpType.mult
    ADD = mybir.AluOpType.add
    nc = bacc.Bacc("TRN2", target_bir_lowering=False, debug=False)

    hT_d = nc.dram_tensor("hT", [DIM, S], mm, kind="ExternalInput").ap()
    w_d = {n: nc.dram_tensor(n, [DIM, CW], mm, kind="ExternalInput").ap()
           for n in ("wq", "wk", "wv")}
    b_d = {n: nc.dram_tensor(n, [1, CW], f32, kind="ExternalInput").ap()
           for n in ("bq", "bk", "bv")}
    A_d = nc.dram_tensor("A", [S, HD], f32, kind="ExternalInput").ap()
    B_d = nc.dram_tensor("B", [S, HD], f32, kind="ExternalInput").ap()
    outN_d = nc.dram_tensor("outN", [HPC, HD + 1, S], f32,
                            kind="ExternalOutput").ap()

    with tile.TileContext(nc) as tc:
        with (
            tc.tile_pool(name="singles", bufs=1) as singles,
            tc.tile_pool(name="hp", bufs=2) as hp,
            tc.tile_pool(name="rp", bufs=3) as rp,
            tc.tile_pool(name="qkp", bufs=3) as qkp,
            tc.tile_pool(name="qrp", bufs=3) as qrp,
            tc.tile_pool(name="vp", bufs=16) as vp,
            tc.tile_pool(name="qtp", bufs=6) as qtp,
            tc.tile_pool(name="ktp", bufs=6) as ktp,
            tc.tile_pool(name="ep", bufs=3) as ep,
            tc.tile_pool(name="op", bufs=3) as op,
            tc.tile_pool(name="ppq", bufs=3, space="PSUM") as ppq,
            tc.tile_pool(name="ppt", bufs=1, space="PSUM") as ppt,
            tc.tile_pool(name="pps", bufs=2, space="PSUM") as pps,
            tc.tile_pool(name="ppv", bufs=2, space="PSUM") as ppv,
        ):
            ident = singles.tile([128, 128], f32, tag="ident", name="ident")
            make_identity(nc, ident)
            w_sb, b_sb = {}, {}
            for n in ("wq", "wk", "wv"):
                t = singles.tile([128, KC * CW], mm, tag=f"{n}_sb",
                                 name=f"{n}_sb")
                nc.sync.dma_start(
                    t.rearrange("p (k c) -> p k c", k=KC),
                    w_d[n].rearrange("(k p) c -> p k c", p=128))
                w_sb[n] = t.rearrange("p (k c) -> p k c", k=KC)
            for n in ("bq", "bk", "bv"):
                t = singles.tile([128, CW], f32, tag=f"{n}_sb", name=f"{n}_sb")
                nc.gpsimd.dma_start(out=t, in_=b_d[n].to_broadcast([128, CW]))
                b_sb[n] = t
            ones3 = singles.tile([128, HPC], f32, tag="ones3", name="ones3")
            nc.vector.memset(ones3, 1.0)

            qT, kT, vt = {}, {}, {}
            pending = []   # attention instances awaiting emission

            def attn_instance(g, h, qh):
                """scoresT -> exp -> PV for one (group, head, query-half),
                software-pipelined over the 8 key chunks."""
                pv = ppv.tile([HD + 1, 512], f32, tag="pv",
                              name=f"pv{g}_{h}_{qh}")
                qs = qT[(g, h)][:, qh * 512:(qh + 1) * 512]

                def exp_pv(kc, st):
                    ex = ep.tile([128, 512], mm, tag="ex",
                                 name=f"ex{g}_{h}_{qh}_{kc}")
                    nc.scalar.activation(ex, st, Exp, scale=SCALE)
                    nc.tensor.matmul(
                        pv, vt[(g, kc)][:, h * 97:(h + 1) * 97], ex,
                        start=(kc == 0), stop=(kc == 7))

                sts = []
                for kc in range(8):
                    st = pps.tile([128, 512], f32, tag="st",
                                  name=f"st{g}_{h}_{qh}_{kc}")
                    nc.tensor.matmul(
                        st, kT[(g, h)][:, kc * 128:(kc + 1) * 128], qs,
                        start=True, stop=True)
                    sts.append(st)
                    if kc >= 1:
                        exp_pv(kc - 1, sts[kc - 1])
                exp_pv(7, sts[7])
                ot = op.tile([HD + 1, 512], f32, tag="ot",
                             name=f"ot{g}_{h}_{qh}")
                nc.scalar.copy(ot, pv)
                nc.sync.dma_start(
                    outN_d[h, :, g * G + qh * 512:g * G + (qh + 1) * 512], ot)

            for blk in range(NB):
                g = blk // 4
                if blk % 4 == 0:
                    for h in range(HPC):
                        qT[(g, h)] = qtp.tile([HD, G], mm, tag="qT",
                                              name=f"qT{g}_{h}")
                        kT[(g, h)] = ktp.tile([HD, G], mm, tag="kT",
                                              name=f"kT{g}_{h}")
                ht = hp.tile([128, KC * TB], mm, tag="ht", name=f"ht{blk}")
                nc.sync.dma_start(
                    ht.rearrange("p (k t) -> p k t", k=KC),
                    hT_d[:, blk * TB:(blk + 1) * TB]
                    .rearrange("(k p) t -> p k t", p=128))
                htv = ht.rearrange("p (k t) -> p k t", k=KC)
                a_t = rp.tile([128, 2 * HD], f32, tag="a", name=f"a{blk}")
                nc.sync.dma_start(
                    a_t.rearrange("p (s c) -> p s c", s=2),
                    A_d[blk * TB:(blk + 1) * TB, :]
                    .rearrange("(s p) c -> p s c", p=128))
                b_t = rp.tile([128, 2 * HD], f32, tag="b", name=f"b{blk}")
                nc.sync.dma_start(
                    b_t.rearrange("p (s c) -> p s c", s=2),
                    B_d[blk * TB:(blk + 1) * TB, :]
                    .rearrange("(s p) c -> p s c", p=128))

                for sub in range(2):
                    tb = blk * 2 + sub
                    col = (tb % 8) * 128
                    a_s = a_t[:, sub * HD:(sub + 1) * HD]
                    b_s = b_t[:, sub * HD:(sub + 1) * HD]
                    # chunk-outer QKV: one stationary hT load serves 3 matmuls
                    ps = {d: ppq.tile([128, CW], f32, tag="ps",
                                      name=f"ps_{d}{tb}")
                          for d in ("q", "k", "v")}
                    for kc in range(KC):
                        lhs = htv[:, kc, sub * 128:(sub + 1) * 128]
                        for n, d in (("wq", "q"), ("wk", "k"), ("wv", "v")):
                            nc.tensor.matmul(ps[d], lhs, w_sb[n][:, kc, :],
                                             start=(kc == 0),
                                             stop=(kc == KC - 1))
                    # V: bias add + interleaved ones column, cast to mm
                    v_t = vp.tile([128, HPC * (HD + 1)], mm, tag="v",
                                  name=f"v{tb}")
                    for h in range(HPC):
                        nc.vector.tensor_tensor(
                            v_t[:, h * 97:h * 97 + 96],
                            ps["v"][:, h * 96:(h + 1) * 96],
                            b_sb["bv"][:, h * 96:(h + 1) * 96], ADD)
                    nc.vector.tensor_copy(
                        v_t.rearrange("p (h c) -> p h c", h=HPC)[:, :, 96:97],
                        ones3.rearrange("p (h c) -> p h c", h=HPC))
                    vt[(g, tb % 8)] = v_t
                    # Q, K: bias, rope, transpose per head
                    for n, d in (("bq", "q"), ("bk", "k")):
                        q_sb = qkp.tile([128, CW], f32, tag=f"{d}sb",
                                        name=f"{d}sb{tb}")
                        nc.vector.tensor_tensor(q_sb, ps[d], b_sb[n], ADD)
                        shf = qkp.tile([128, CW], f32, tag="shf",
                                       name=f"shf_{d}{tb}")
                        qv = q_sb.rearrange("p (h c u f) -> p h c u f",
                                            h=3, c=3, u=2)
                        sv = shf.rearrange("p (h c u f) -> p h c u f",
                                           h=3, c=3, u=2)
                        nc.vector.tensor_copy(sv[:, :, :, 0:1, :],
                                              qv[:, :, :, 1:2, :])
                        nc.vector.tensor_copy(sv[:, :, :, 1:2, :],
                                              qv[:, :, :, 0:1, :])
                        qr = qrp.tile([128, CW], f32, tag="qr",
                                      name=f"qr_{d}{tb}")
                        for h in range(HPC):
                            hs_ = slice(h * 96, (h + 1) * 96)
                            nc.vector.tensor_tensor(shf[:, hs_], shf[:, hs_],
                                                    b_s, MUL)
                            nc.vector.tensor_tensor(q_sb[:, hs_], q_sb[:, hs_],
                                                    a_s, MUL)
                        nc.vector.tensor_tensor(qr, q_sb, shf, ADD)
                        dst = qT if d == "q" else kT
                        for h in range(HPC):
                            pt = ppt.tile([HD, 128], f32, tag="pt",
                                          name=f"pt_{d}{tb}_{h}")
                            nc.tensor.transpose(
                                pt, qr[:, h * 96:(h + 1) * 96], ident)
                            nc.scalar.copy(dst[(g, h)][:, col:col + 128], pt)
                    # drain one pending attention instance per sub-tile
                    if pending:
                        attn_instance(*pending.pop(0))
                if blk % 4 == 3:
                    pending.extend((g, h, qh)
                                   for h in range(HPC) for qh in range(2))
            while pending:
                attn_instance(*pending.pop(0))
    nc.compile()
    return nc


def _build_launch2():
    import concourse.mybir as mybir
    import concourse.tile as tile
    from concourse import bacc

    f32 = mybir.dt.float32
    mm = _mm_dt()
    TOK = 1024           # tokens per core
    NW = 1152            # outdims per core
    MB = NW // 128       # 9 outdim blocks
    nc = bacc.Bacc("TRN2", target_bir_lowering=False, debug=False)

    at_d = nc.dram_tensor("attnT", [DIM, TOK], mm, kind="ExternalInput").ap()
    wo_d = nc.dram_tensor("woj", [DIM, NW], mm, kind="ExternalInput").ap()
    bo_d = nc.dram_tensor("boj", [1, NW], f32, kind="ExternalInput").ap()
    # transposed output [outdim, tok]; host transposes back
    out_d = nc.dram_tensor("out", [NW, TOK], f32, kind="ExternalOutput").ap()

    with tile.TileContext(nc) as tc:
        ats, wos = [], []
        with (
            tc.tile_pool(name="singles2", bufs=1) as singles,
            tc.tile_pool(name="atp", bufs=KC) as atp,
            tc.tile_pool(name="wop", bufs=KC) as wop,
            tc.tile_pool(name="outp", bufs=3) as outp,
            tc.tile_pool(name="psp", bufs=4, space="PSUM") as psp,
        ):
            bo_sb = singles.tile([128, MB], f32, tag="bo_sb", name="bo_sb")
            nc.sync.dma_start(bo_sb,
                              bo_d.rearrange("a (m p) -> p (a m)", p=128))
            for kc in range(KC):
                a = atp.tile([128, TOK], mm, tag="at", name=f"at{kc}")
                nc.sync.dma_start(a, at_d[kc * 128:(kc + 1) * 128, :])
                ats.append(a)
                w = wop.tile([128, NW], mm, tag="wo", name=f"wo{kc}")
                nc.sync.dma_start(w, wo_d[kc * 128:(kc + 1) * 128, :])
                wos.append(w)
            for mb in range(MB):
                ot = outp.tile([128, TOK], f32, tag="ot", name=f"ot{mb}")
                pss = [psp.tile([128, 512], f32, tag="ps", name=f"ps{mb}_{th}")
                       for th in range(2)]
                for kc in range(KC):
                    lhs = wos[kc][:, mb * 128:(mb + 1) * 128]
                    for th in range(2):
                        nc.tensor.matmul(
                            pss[th], lhs, ats[kc][:, th * 512:(th + 1) * 512],
                            start=(kc == 0), stop=(kc == KC - 1))
                for th in range(2):
                    nc.vector.tensor_scalar_add(
                        ot[:, th * 512:(th + 1) * 512], pss[th],
                        bo_sb[:, mb:mb + 1])
                nc.sync.dma_start(out_d[mb * 128:(mb + 1) * 128, :], ot)
    nc.compile()
    return nc


def _get(name, builder):
    if name not in _CACHE:
        _CACHE[name] = builder()
    return _CACHE[name]


def _rope_tables(frame, height, width):
    t = np.repeat(np.arange(frame), height * width)
    y = np.tile(np.repeat(np.arange(height), width), frame)
    x = np.tile(np.arange(width), frame * height)
    D = HD // 3
    A = np.empty((S, HD), np.float32)
    B = np.empty((S, HD), np.float32)
    for i, pos in enumerate((t, y, x)):
        inv = 1.0 / (10000.0 ** (np.arange(0, D, 2, dtype=np.float32) / D))
        f = pos[:, None].astype(np.float32) * inv[None, :]
        A[:, i * D:i * D + 16] = np.cos(f)
        A[:, i * D + 16:(i + 1) * D] = np.cos(f)
        B[:, i * D:i * D + 16] = -np.sin(f)
        B[:, i * D + 16:(i + 1) * D] = np.sin(f)
    return A, B


def kernel(hidden_states, wq, bq, wk, bk, wv, bv, wo, bo, frame, height, width):
    from concourse import bass_utils

    f, hh, ww = int(frame), int(height), int(width)
    hs = np.asarray(hidden_states, dtype=np.float32)
    assert hs.shape == (1, S, DIM) and f * hh * ww == S
    wq, wk, wv, wo = (np.asarray(a, np.float32) for a in (wq, wk, wv, wo))
    bq, bk, bv, bo = (np.asarray(a, np.float32) for a in (bq, bk, bv, bo))

    perm = np.concatenate([np.arange(k, S, SPN) for k in range(SPN)])
    A, B = _rope_tables(f, hh, ww)
    A = np.ascontiguousarray(A[perm])
    B = np.ascontiguousarray(B[perm])
    hT = np.ascontiguousarray(hs[0].T[:, perm])

    nc1 = _get("l1", _build_launch1)
    in1 = []
    for c in range(8):
        sl = slice(c * CW, (c + 1) * CW)
        in1.append({
            "hT": hT,
            "wq": np.ascontiguousarray(wq[:, sl]),
            "wk": np.ascontiguousarray(wk[:, sl]),
            "wv": np.ascontiguousarray(wv[:, sl]),
            "bq": np.ascontiguousarray(bq[sl]).reshape(1, CW),
            "bk": np.ascontiguousarray(bk[sl]).reshape(1, CW),
            "bv": np.ascontiguousarray(bv[sl]).reshape(1, CW),
            "A": A, "B": B,
        })
    res1 = bass_utils.run_bass_kernel_spmd(nc1, in1, core_ids=list(range(8)))
    LAST_RESULTS.append(res1)

    outN = np.concatenate([res1.results[c]["outN"] for c in range(8)], 0)
    attnT_g = (outN[:, :HD, :] / outN[:, HD:HD + 1, :]).reshape(DIM, S)
    attnT = np.empty_like(attnT_g)
    attnT[:, perm] = attnT_g

    nc2 = _get("l2", _build_launch2)
    in2 = []
    for c in range(8):
        i, j = divmod(c, 2)
        in2.append({
            "attnT": np.ascontiguousarray(attnT[:, i * 1024:(i + 1) * 1024]),
            "woj": np.ascontiguousarray(wo[:, j * 1152:(j + 1) * 1152]),
            "boj": np.ascontiguousarray(bo[j * 1152:(j + 1) * 1152]).reshape(1, 1152),
        })
    res2 = bass_utils.run_bass_kernel_spmd(nc2, in2, core_ids=list(range(8)))
    LAST_RESULTS.append(res2)

    out = np.empty((S, DIM), np.float32)
    for c in range(8):
        i, j = divmod(c, 2)
        out[i * 1024:(i + 1) * 1024, j * 1152:(j + 1) * 1152] = \
            res2.results[c]["out"].T
    return out[None]


# revision 10
# speedup vs baseline: 2.9722x; 1.0737x over previous
"""Trainium2 Bass kernel for nn_BasicTransformerBlock_18657337934637.

Sparse-attention transformer block:
  q/k/v = hidden @ W* + b*        (2304 -> 2304, 24 heads x 96)
  RoPE3D on q, k
  sparse-1d grouping (SPARSE_N=4): token t -> group t%4, 1024 tokens/group
  softmax attention within each (group, head)
  out = attn @ wo + bo

Distribution over 8 NeuronCores:
  Launch 1 (head-parallel): core c computes heads 3c..3c+2 end-to-end through
    attention.  Host pre-transposes hidden to hT [2304, 4096] in grouped token
    order, so groups are contiguous 1024-token spans and the QKV matmuls need
    no on-device transpose of the activations.  Per (group, head): scores are
    computed transposed [k, q] so softmax-exp sums and the P@V contraction both
    run with k on the partition axis; an all-ones column appended to v yields
    the softmax denominator for free in the same matmul; exp skips the max
    subtraction (scores are O(5), fp32 exp is safe).  Output: un-normalized
    attn^T + denominator row, [3, 97, 4096] per core; the host divides.
  Host: gather heads -> attnT [2304, 4096], undo token permutation.
  Launch 2 (token x outdim parallel): core (i, j) computes
    out[i*1024:(i+1)*1024, j*1152:(j+1)*1152]^T = wo_j^T @ attnT_i
    (output kept transposed so the weight stays stationary on the PE).

Matmuls run as float32r (full fp32 storage; TensorE single-pass mode, 1 row/cyc
for moving dim >= 256).  Set KERNEL_MM_DT=f32 to fall back to exact-fp32
two-pass matmuls.
"""
import os
import numpy as np

HEADS = 24
HD = 96
SPN = 4
S = 4096
DIM = 2304
KC = DIM // 128            # 18 contraction chunks
HPC = 3                    # heads per core
CW = HPC * HD              # 288 columns per core
G = S // SPN               # 1024 tokens per group
TB = 256                   # hT dma block (tokens)
NB = S // TB               # 16 blocks
SCALE = 1.0 / float(np.sqrt(HD))

_CACHE = {}
LAST_RESULTS = []          # test harness introspection


def _mm_dt():
    import concourse.mybir as mybir
    return (mybir.dt.float32 if os.environ.get("KERNEL_MM_DT") == "f32"
            else mybir.dt.float32r)


def _build_launch1():
    import concourse.mybir as mybir
    import concourse.tile as tile
    from concourse import bacc
    from concourse.masks import make_identity

    f32 = mybir.dt.float32
    mm = _mm_dt()
    Exp = mybir.ActivationFunctionType.Exp
    MUL = mybir.AluOpType.mult
    ADD = mybir.AluOpType.add
    nc = bacc.Bacc("TRN2", target_bir_lowering=False, debug=False)

    hT_d = nc.dram_tensor("hT", [DIM, S], mm, kind="ExternalInput").ap()
    w_d = {n: nc.dram_tensor(n, [DIM, CW], mm, kind="ExternalInput").ap()
           for n in ("wq", "wk", "wv")}
    b_d = {n: nc.dram_tensor(n, [1, CW], f32, kind="ExternalInput").ap()
           for n in ("bq", "bk", "bv")}
    A_d = nc.dram_tensor("A", [S, HD], f32, kind="ExternalInput").ap()
    B_d = nc.dram_tensor("B", [S, HD], f32, kind="ExternalInput").ap()
    outN_d = nc.dram_tensor("outN", [HPC, HD + 1, S], f32,
                            kind="ExternalOutput").ap()

    with tile.TileContext(nc) as tc:
        with (
            tc.tile_pool(name="singles", bufs=1) as singles,
            tc.tile_pool(name="hp", bufs=2) as hp,
            tc.tile_pool(name="rp", bufs=3) as rp,
            tc.tile_pool(name="qkp", bufs=3) as qkp,
            tc.tile_pool(name="qrp", bufs=3) as qrp,
            tc.tile_pool(name="vp", bufs=16) as vp,
            tc.tile_pool(name="qtp", bufs=6) as qtp,
            tc.tile_pool(name="ktp", bufs=6) as ktp,
            tc.tile_pool(name="ep", bufs=3) as ep,
            tc.tile_pool(name="op", bufs=3) as op,
            tc.tile_pool(name="ppq", bufs=3, space="PSUM") as ppq,
            tc.tile_pool(name="ppt", bufs=1, space="PSUM") as ppt,
            tc.tile_pool(name="pps", bufs=2, space="PSUM") as pps,
            tc.tile_pool(name="ppv", bufs=2, space="PSUM") as ppv,
        ):
            ident = singles.tile([128, 128], f32, tag="ident", name="ident")
            make_identity(nc, ident)
            w_sb, b_sb = {}, {}
            for n in ("wq", "wk", "wv"):
                t = singles.tile([128, KC * CW], mm, tag=f"{n}_sb",
                                 name=f"{n}_sb")
                nc.sync.dma_start(
                    t.rearrange("p (k c) -> p k c", k=KC),
                    w_d[n].rearrange("(k p) c -> p k c", p=128))
                w_sb[n] = t.rearrange("p (k c) -> p k c", k=KC)
            for n in ("bq", "bk", "bv"):
                t = singles.tile([128, CW], f32, tag=f"{n}_sb", name=f"{n}_sb")
                nc.gpsimd.dma_start(out=t, in_=b_d[n].to_broadcast([128, CW]))
                b_sb[n] = t
            ones3 = singles.tile([128, HPC], f32, tag="ones3", name="ones3")
            nc.vector.memset(ones3, 1.0)

            qT, kT, vt = {}, {}, {}
            pending = []   # attention instances awaiting emission

            def attn_instance(g, h, qh):
                """scoresT -> exp -> PV for one (group, head, query-half),
                software-pipelined over the 8 key chunks."""
                pv = ppv.tile([HD + 1, 512], f32, tag="pv",
                              name=f"pv{g}_{h}_{qh}")
                qs = qT[(g, h)][:, qh * 512:(qh + 1) * 512]

                def exp_pv(kc, st):
                    ex = ep.tile([128, 512], mm, tag="ex",
                                 name=f"ex{g}_{h}_{qh}_{kc}")
                    nc.scalar.activation(ex, st, Exp, scale=SCALE)
                    nc.tensor.matmul(
                        pv, vt[(g, kc)][:, h * 97:(h + 1) * 97], ex,
                        start=(kc == 0), stop=(kc == 7))

                sts = []
                for kc in range(8):
                    st = pps.tile([128, 512], f32, tag="st",
                                  name=f"st{g}_{h}_{qh}_{kc}")
                    nc.tensor.matmul(
                        st, kT[(g, h)][:, kc * 128:(kc + 1) * 128], qs,
                        start=True, stop=True)
                    sts.append(st)
                    if kc >= 1:
                        exp_pv(kc - 1, sts[kc - 1])
                exp_pv(7, sts[7])
                ot = op.tile([HD + 1, 512], f32, tag="ot",
                             name=f"ot{g}_{h}_{qh}")
                nc.scalar.copy(ot, pv)
                nc.sync.dma_start(
                    outN_d[h, :, g * G + qh * 512:g * G + (qh + 1) * 512], ot)

            for blk in range(NB):
                g = blk // 4
                if blk % 4 == 0:
                    for h in range(HPC):
                        qT[(g, h)] = qtp.tile([HD, G], mm, tag="qT",
                                              name=f"qT{g}_{h}")
                        kT[(g, h)] = ktp.tile([HD, G], mm, tag="kT",
                                              name=f"kT{g}_{h}")
                ht = hp.tile([128, KC * TB], mm, tag="ht", name=f"ht{blk}")
                nc.sync.dma_start(
                    ht.rearrange("p (k t) -> p k t", k=KC),
                    hT_d[:, blk * TB:(blk + 1) * TB]
                    .rearrange("(k p) t -> p k t", p=128))
                htv = ht.rearrange("p (k t) -> p k t", k=KC)
                a_t = rp.tile([128, 2 * HD], f32, tag="a", name=f"a{blk}")
                nc.sync.dma_start(
                    a_t.rearrange("p (s c) -> p s c", s=2),
                    A_d[blk * TB:(blk + 1) * TB, :]
                    .rearrange("(s p) c -> p s c", p=128))
                b_t = rp.tile([128, 2 * HD], f32, tag="b", name=f"b{blk}")
                nc.sync.dma_start(
                    b_t.rearrange("p (s c) -> p s c", s=2),
                    B_d[blk * TB:(blk + 1) * TB, :]
                    .rearrange("(s p) c -> p s c", p=128))

                for sub in range(2):
                    tb = blk * 2 + sub
                    col = (tb % 8) * 128
                    a_s = a_t[:, sub * HD:(sub + 1) * HD]
                    b_s = b_t[:, sub * HD:(sub + 1) * HD]
                    # chunk-outer QKV: one stationary hT load serves 3 matmuls
                    ps = {d: ppq.tile([128, CW], f32, tag="ps",
                                      name=f"ps_{d}{tb}")
                          for d in ("q", "k", "v")}
                    for kc in range(KC):
                        lhs = htv[:, kc, sub * 128:(sub + 1) * 128]
                        for n, d in (("wq", "q"), ("wk", "k"), ("wv", "v")):
                            nc.tensor.matmul(ps[d], lhs, w_sb[n][:, kc, :],
                                             start=(kc == 0),
                                             stop=(kc == KC - 1))
                    # V: bias add + interleaved ones column, cast to mm
                    v_t = vp.tile([128, HPC * (HD + 1)], mm, tag="v",
                                  name=f"v{tb}")
                    for h in range(HPC):
                        nc.vector.tensor_tensor(
                            v_t[:, h * 97:h * 97 + 96],
                            ps["v"][:, h * 96:(h + 1) * 96],
                            b_sb["bv"][:, h * 96:(h + 1) * 96], ADD)
                    nc.vector.tensor_copy(
                        v_t.rearrange("p (h c) -> p h c", h=HPC)[:, :, 96:97],
                        ones3.rearrange("p (h c) -> p h c", h=HPC))
                    vt[(g, tb % 8)] = v_t
                    # Q, K: bias, rope, transpose per head
                    for n, d in (("bq", "q"), ("bk", "k")):
                        q_sb = qkp.tile([128, CW], f32, tag=f"{d}sb",
                                        name=f"{d}sb{tb}")
                        nc.vector.tensor_tensor(q_sb, ps[d], b_sb[n], ADD)
                        shf = qkp.tile([128, CW], f32, tag="shf",
                                       name=f"shf_{d}{tb}")
                        qv = q_sb.rearrange("p (h c u f) -> p h c u f",
                                            h=3, c=3, u=2)
                        sv = shf.rearrange("p (h c u f) -> p h c u f",
                                           h=3, c=3, u=2)
                        nc.vector.tensor_copy(sv[:, :, :, 0:1, :],
                                              qv[:, :, :, 1:2, :])
                        nc.vector.tensor_copy(sv[:, :, :, 1:2, :],
                                              qv[:, :, :, 0:1, :])
                        qr = qrp.tile([128, CW], f32, tag="qr",
                                      name=f"qr_{d}{tb}")
                        for h in range(HPC):
                            hs_ = slice(h * 96, (h + 1) * 96)
                            nc.vector.tensor_tensor(shf[:, hs_], shf[:, hs_],
                                                    b_s, MUL)
                            nc.vector.tensor_tensor(q_sb[:, hs_], q_sb[:, hs_],
                                                    a_s, MUL)
                        nc.vector.tensor_tensor(qr, q_sb, shf, ADD)
                        dst = qT if d == "q" else kT
                        for h in range(HPC):
                            pt = ppt.tile([HD, 128], f32, tag="pt",
                                          name=f"pt_{d}{tb}_{h}")
                            nc.tensor.transpose(
                                pt, qr[:, h * 96:(h + 1) * 96], ident)
                            nc.scalar.copy(dst[(g, h)][:, col:col + 128], pt)
                    # drain one pending attention instance per sub-tile
                    if pending:
                        attn_instance(*pending.pop(0))
                if blk % 4 == 3:
                    pending.extend((g, h, qh)
                                   for h in range(HPC) for qh in range(2))
            while pending:
                attn_instance(*pending.pop(0))
    nc.compile()
    return nc


def _build_launch2():
    import concourse.mybir as mybir
    import concourse.tile as tile
    from concourse import bacc

    f32 = mybir.dt.float32
    bf16 = os.environ.get("KERNEL_L2_BF16", "1") == "1"
    mm = mybir.dt.bfloat16 if bf16 else _mm_dt()
    TOK = 1024           # tokens per core
    NW = 1152            # outdims per core
    MB = NW // 128       # 9 outdim blocks
    nc = bacc.Bacc("TRN2", target_bir_lowering=False, debug=False)

    at_d = nc.dram_tensor("attnT", [DIM, TOK], mm, kind="ExternalInput").ap()
    wo_d = nc.dram_tensor("woj", [DIM, NW], mm, kind="ExternalInput").ap()
    bo_d = nc.dram_tensor("boj", [1, NW], f32, kind="ExternalInput").ap()
    # transposed output [outdim, tok]; host transposes back
    out_d = nc.dram_tensor("out", [NW, TOK], f32, kind="ExternalOutput").ap()

    with tile.TileContext(nc) as tc:
        ats, wos = [], []
        with (
            tc.tile_pool(name="singles2", bufs=1) as singles,
            tc.tile_pool(name="atp", bufs=KC) as atp,
            tc.tile_pool(name="wop", bufs=KC) as wop,
            tc.tile_pool(name="outp", bufs=4) as outp,
            tc.tile_pool(name="psp", bufs=8, space="PSUM") as psp,
        ):
            bo_sb = singles.tile([128, MB], f32, tag="bo_sb", name="bo_sb")
            nc.sync.dma_start(bo_sb,
                              bo_d.rearrange("a (m p) -> p (a m)", p=128))
            for kc in range(KC):
                a = atp.tile([128, TOK], mm, tag="at", name=f"at{kc}")
                nc.sync.dma_start(a, at_d[kc * 128:(kc + 1) * 128, :])
                ats.append(a)
                w = wop.tile([128, NW], mm, tag="wo", name=f"wo{kc}")
                nc.sync.dma_start(w, wo_d[kc * 128:(kc + 1) * 128, :])
                wos.append(w)
            # chunk-outer accumulation over groups of 4 outdim blocks
            # (8 psum banks per group) so the PE tracks the DMA feed instead
            # of serializing behind it.
            units = [(mb, th) for mb in range(MB) for th in range(2)]
            ots = {}
            for base in range(0, len(units), 8):
                grp = units[base:base + 8]
                pss = {}
                for mb, th in grp:
                    pss[(mb, th)] = psp.tile([128, 512], f32, tag="ps",
                                             name=f"ps{mb}_{th}")
                for kc in range(KC):
                    for mb, th in grp:
                        nc.tensor.matmul(
                            pss[(mb, th)], wos[kc][:, mb * 128:(mb + 1) * 128],
                            ats[kc][:, th * 512:(th + 1) * 512],
                            start=(kc == 0), stop=(kc == KC - 1))
                for mb, th in grp:
                    if mb not in ots:
                        ots[mb] = outp.tile([128, TOK], f32, tag="ot",
                                            name=f"ot{mb}")
                    nc.vector.tensor_scalar_add(
                        ots[mb][:, th * 512:(th + 1) * 512], pss[(mb, th)],
                        bo_sb[:, mb:mb + 1])
                    if th == 1:
                        nc.sync.dma_start(out_d[mb * 128:(mb + 1) * 128, :],
                                          ots[mb])
    nc.compile()
    return nc


def _get(name, builder):
    if name not in _CACHE:
        _CACHE[name] = builder()
    return _CACHE[name]


def _rope_tables(frame, height, width):
    t = np.repeat(np.arange(frame), height * width)
    y = np.tile(np.repeat(np.arange(height), width), frame)
    x = np.tile(np.arange(width), frame * height)
    D = HD // 3
    A = np.empty((S, HD), np.float32)
    B = np.empty((S, HD), np.float32)
    for i, pos in enumerate((t, y, x)):
        inv = 1.0 / (10000.0 ** (np.arange(0, D, 2, dtype=np.float32) / D))
        f = pos[:, None].astype(np.float32) * inv[None, :]
        A[:, i * D:i * D + 16] = np.cos(f)
        A[:, i * D + 16:(i + 1) * D] = np.cos(f)
        B[:, i * D:i * D + 16] = -np.sin(f)
        B[:, i * D + 16:(i + 1) * D] = np.sin(f)
    return A, B


def kernel(hidden_states, wq, bq, wk, bk, wv, bv, wo, bo, frame, height, width):
    from concourse import bass_utils

    f, hh, ww = int(frame), int(height), int(width)
    hs = np.asarray(hidden_states, dtype=np.float32)
    assert hs.shape == (1, S, DIM) and f * hh * ww == S
    wq, wk, wv, wo = (np.asarray(a, np.float32) for a in (wq, wk, wv, wo))
    bq, bk, bv, bo = (np.asarray(a, np.float32) for a in (bq, bk, bv, bo))

    perm = np.concatenate([np.arange(k, S, SPN) for k in range(SPN)])
    A, B = _rope_tables(f, hh, ww)
    A = np.ascontiguousarray(A[perm])
    B = np.ascontiguousarray(B[perm])
    hT = np.ascontiguousarray(hs[0].T[:, perm])

    nc1 = _get("l1", _build_launch1)
    in1 = []
    for c in range(8):
        sl = slice(c * CW, (c + 1) * CW)
        in1.append({
            "hT": hT,
            "wq": np.ascontiguousarray(wq[:, sl]),
            "wk": np.ascontiguousarray(wk[:, sl]),
            "wv": np.ascontiguousarray(wv[:, sl]),
            "bq": np.ascontiguousarray(bq[sl]).reshape(1, CW),
            "bk": np.ascontiguousarray(bk[sl]).reshape(1, CW),
            "bv": np.ascontiguousarray(bv[sl]).reshape(1, CW),
            "A": A, "B": B,
        })
    res1 = bass_utils.run_bass_kernel_spmd(nc1, in1, core_ids=list(range(8)))
    LAST_RESULTS.append(res1)

    outN = np.concatenate([res1.results[c]["outN"] for c in range(8)], 0)
    attnT_g = (outN[:, :HD, :] / outN[:, HD:HD + 1, :]).reshape(DIM, S)
    attnT = np.empty_like(attnT_g)
    attnT[:, perm] = attnT_g

    nc2 = _get("l2", _build_launch2)
    if os.environ.get("KERNEL_L2_BF16", "1") == "1":
        import ml_dtypes
        l2dt = ml_dtypes.bfloat16
    else:
        l2dt = np.float32
    in2 = []
    for c in range(8):
        i, j = divmod(c, 2)
        in2.append({
            "attnT": np.ascontiguousarray(
                attnT[:, i * 1024:(i + 1) * 1024].astype(l2dt)),
            "woj": np.ascontiguousarray(
                wo[:, j * 1152:(j + 1) * 1152].astype(l2dt)),
            "boj": np.ascontiguousarray(bo[j * 1152:(j + 1) * 1152]).reshape(1, 1152),
        })
    res2 = bass_utils.run_bass_kernel_spmd(nc2, in2, core_ids=list(range(8)))
    LAST_RESULTS.append(res2)

    out = np.empty((S, DIM), np.float32)
    for c in range(8):
        i, j = divmod(c, 2)
        out[i * 1024:(i + 1) * 1024, j * 1152:(j + 1) * 1152] = \
            res2.results[c]["out"].T
    return out[None]


# revision 11
# speedup vs baseline: 3.0053x; 1.0111x over previous
"""Trainium2 Bass kernel for nn_BasicTransformerBlock_18657337934637.

Sparse-attention transformer block:
  q/k/v = hidden @ W* + b*        (2304 -> 2304, 24 heads x 96)
  RoPE3D on q, k
  sparse-1d grouping (SPARSE_N=4): token t -> group t%4, 1024 tokens/group
  softmax attention within each (group, head)
  out = attn @ wo + bo

Distribution over 8 NeuronCores:
  Launch 1 (head-parallel): core c computes heads 3c..3c+2 end-to-end through
    attention.  Host pre-transposes hidden to hT [2304, 4096] in grouped token
    order, so groups are contiguous 1024-token spans and the QKV matmuls need
    no on-device transpose of the activations.  Per (group, head): scores are
    computed transposed [k, q] so softmax-exp sums and the P@V contraction both
    run with k on the partition axis; an all-ones column appended to v yields
    the softmax denominator for free in the same matmul; exp skips the max
    subtraction (scores are O(5), fp32 exp is safe).  Output: un-normalized
    attn^T + denominator row, [3, 97, 4096] per core; the host divides.
  Host: gather heads -> attnT [2304, 4096], undo token permutation.
  Launch 2 (token x outdim parallel): core (i, j) computes
    out[i*1024:(i+1)*1024, j*1152:(j+1)*1152]^T = wo_j^T @ attnT_i
    (output kept transposed so the weight stays stationary on the PE).

Matmuls run as float32r (full fp32 storage; TensorE single-pass mode, 1 row/cyc
for moving dim >= 256).  Set KERNEL_MM_DT=f32 to fall back to exact-fp32
two-pass matmuls.
"""
import os
import numpy as np

HEADS = 24
HD = 96
SPN = 4
S = 4096
DIM = 2304
KC = DIM // 128            # 18 contraction chunks
HPC = 3                    # heads per core
CW = HPC * HD              # 288 columns per core
G = S // SPN               # 1024 tokens per group
TB = 256                   # hT dma block (tokens)
NB = S // TB               # 16 blocks
SCALE = 1.0 / float(np.sqrt(HD))

_CACHE = {}
LAST_RESULTS = []          # test harness introspection


def _mm_dt():
    import concourse.mybir as mybir
    return (mybir.dt.float32 if os.environ.get("KERNEL_MM_DT") == "f32"
            else mybir.dt.float32r)


def _build_launch1():
    import concourse.mybir as mybir
    import concourse.tile as tile
    from concourse import bacc
    from concourse.masks import make_identity

    f32 = mybir.dt.float32
    mm = _mm_dt()
    Exp = mybir.ActivationFunctionType.Exp
    MUL = mybir.AluOpType.mult
    ADD = mybir.AluOpType.add
    nc = bacc.Bacc("TRN2", target_bir_lowering=False, debug=False)

    # all inputs host-pre-tiled to the exact SBUF layouts -> every DMA is a
    # plain 2D copy with multi-KB contiguous rows (full HBM bandwidth)
    hT_d = nc.dram_tensor("hT", [NB, 128, KC * TB], mm,
                          kind="ExternalInput").ap()
    w_d = {n: nc.dram_tensor(n, [128, KC * CW], mm, kind="ExternalInput").ap()
           for n in ("wq", "wk", "wv")}
    b_d = {n: nc.dram_tensor(n, [1, CW], f32, kind="ExternalInput").ap()
           for n in ("bq", "bk", "bv")}
    A_d = nc.dram_tensor("A", [NB, 128, 2 * HD], f32, kind="ExternalInput").ap()
    B_d = nc.dram_tensor("B", [NB, 128, 2 * HD], f32, kind="ExternalInput").ap()
    outN_d = nc.dram_tensor("outN", [HPC, HD + 1, S], f32,
                            kind="ExternalOutput").ap()

    with tile.TileContext(nc) as tc:
        with (
            tc.tile_pool(name="singles", bufs=1) as singles,
            tc.tile_pool(name="hp", bufs=2) as hp,
            tc.tile_pool(name="rp", bufs=3) as rp,
            tc.tile_pool(name="qkp", bufs=3) as qkp,
            tc.tile_pool(name="qrp", bufs=3) as qrp,
            tc.tile_pool(name="vp", bufs=16) as vp,
            tc.tile_pool(name="qtp", bufs=6) as qtp,
            tc.tile_pool(name="ktp", bufs=6) as ktp,
            tc.tile_pool(name="ep", bufs=3) as ep,
            tc.tile_pool(name="op", bufs=3) as op,
            tc.tile_pool(name="ppq", bufs=3, space="PSUM") as ppq,
            tc.tile_pool(name="ppt", bufs=1, space="PSUM") as ppt,
            tc.tile_pool(name="pps", bufs=2, space="PSUM") as pps,
            tc.tile_pool(name="ppv", bufs=2, space="PSUM") as ppv,
        ):
            ident = singles.tile([128, 128], f32, tag="ident", name="ident")
            make_identity(nc, ident)
            w_sb, b_sb = {}, {}
            for n in ("wq", "wk", "wv"):
                t = singles.tile([128, KC * CW], mm, tag=f"{n}_sb",
                                 name=f"{n}_sb")
                nc.sync.dma_start(t, w_d[n])
                w_sb[n] = t.rearrange("p (k c) -> p k c", k=KC)
            for n in ("bq", "bk", "bv"):
                t = singles.tile([128, CW], f32, tag=f"{n}_sb", name=f"{n}_sb")
                nc.gpsimd.dma_start(out=t, in_=b_d[n].to_broadcast([128, CW]))
                b_sb[n] = t
            ones3 = singles.tile([128, HPC], f32, tag="ones3", name="ones3")
            nc.vector.memset(ones3, 1.0)

            qT, kT, vt = {}, {}, {}
            pending = []   # attention instances awaiting emission

            def attn_instance(g, h, qh):
                """scoresT -> exp -> PV for one (group, head, query-half),
                software-pipelined over the 8 key chunks."""
                pv = ppv.tile([HD + 1, 512], f32, tag="pv",
                              name=f"pv{g}_{h}_{qh}")
                qs = qT[(g, h)][:, qh * 512:(qh + 1) * 512]

                def exp_pv(kc, st):
                    ex = ep.tile([128, 512], mm, tag="ex",
                                 name=f"ex{g}_{h}_{qh}_{kc}")
                    nc.scalar.activation(ex, st, Exp, scale=SCALE)
                    nc.tensor.matmul(
                        pv, vt[(g, kc)][:, h * 97:(h + 1) * 97], ex,
                        start=(kc == 0), stop=(kc == 7))

                sts = []
                for kc in range(8):
                    st = pps.tile([128, 512], f32, tag="st",
                                  name=f"st{g}_{h}_{qh}_{kc}")
                    nc.tensor.matmul(
                        st, kT[(g, h)][:, kc * 128:(kc + 1) * 128], qs,
                        start=True, stop=True)
                    sts.append(st)
                    if kc >= 1:
                        exp_pv(kc - 1, sts[kc - 1])
                exp_pv(7, sts[7])
                ot = op.tile([HD + 1, 512], f32, tag="ot",
                             name=f"ot{g}_{h}_{qh}")
                nc.scalar.copy(ot, pv)
                nc.scalar.dma_start(
                    outN_d[h, :, g * G + qh * 512:g * G + (qh + 1) * 512], ot)

            for blk in range(NB):
                g = blk // 4
                if blk % 4 == 0:
                    for h in range(HPC):
                        qT[(g, h)] = qtp.tile([HD, G], mm, tag="qT",
                                              name=f"qT{g}_{h}")
                        kT[(g, h)] = ktp.tile([HD, G], mm, tag="kT",
                                              name=f"kT{g}_{h}")
                ht = hp.tile([128, KC * TB], mm, tag="ht", name=f"ht{blk}")
                nc.sync.dma_start(ht, hT_d[blk])
                htv = ht.rearrange("p (k t) -> p k t", k=KC)
                a_t = rp.tile([128, 2 * HD], f32, tag="a", name=f"a{blk}")
                nc.scalar.dma_start(a_t, A_d[blk])
                b_t = rp.tile([128, 2 * HD], f32, tag="b", name=f"b{blk}")
                nc.scalar.dma_start(b_t, B_d[blk])

                for sub in range(2):
                    tb = blk * 2 + sub
                    col = (tb % 8) * 128
                    a_s = a_t[:, sub * HD:(sub + 1) * HD]
                    b_s = b_t[:, sub * HD:(sub + 1) * HD]
                    # chunk-outer QKV: one stationary hT load serves 3 matmuls
                    ps = {d: ppq.tile([128, CW], f32, tag="ps",
                                      name=f"ps_{d}{tb}")
                          for d in ("q", "k", "v")}
                    for kc in range(KC):
                        lhs = htv[:, kc, sub * 128:(sub + 1) * 128]
                        for n, d in (("wq", "q"), ("wk", "k"), ("wv", "v")):
                            nc.tensor.matmul(ps[d], lhs, w_sb[n][:, kc, :],
                                             start=(kc == 0),
                                             stop=(kc == KC - 1))
                    # V: bias add + interleaved ones column, cast to mm
                    v_t = vp.tile([128, HPC * (HD + 1)], mm, tag="v",
                                  name=f"v{tb}")
                    for h in range(HPC):
                        nc.vector.tensor_tensor(
                            v_t[:, h * 97:h * 97 + 96],
                            ps["v"][:, h * 96:(h + 1) * 96],
                            b_sb["bv"][:, h * 96:(h + 1) * 96], ADD)
                    nc.vector.tensor_copy(
                        v_t.rearrange("p (h c) -> p h c", h=HPC)[:, :, 96:97],
                        ones3.rearrange("p (h c) -> p h c", h=HPC))
                    vt[(g, tb % 8)] = v_t
                    # Q, K: bias, rope, transpose per head
                    for n, d in (("bq", "q"), ("bk", "k")):
                        q_sb = qkp.tile([128, CW], f32, tag=f"{d}sb",
                                        name=f"{d}sb{tb}")
                        nc.vector.tensor_tensor(q_sb, ps[d], b_sb[n], ADD)
                        shf = qkp.tile([128, CW], f32, tag="shf",
                                       name=f"shf_{d}{tb}")
                        qv = q_sb.rearrange("p (h c u f) -> p h c u f",
                                            h=3, c=3, u=2)
                        sv = shf.rearrange("p (h c u f) -> p h c u f",
                                           h=3, c=3, u=2)
                        nc.vector.tensor_copy(sv[:, :, :, 0:1, :],
                                              qv[:, :, :, 1:2, :])
                        nc.vector.tensor_copy(sv[:, :, :, 1:2, :],
                                              qv[:, :, :, 0:1, :])
                        qr = qrp.tile([128, CW], f32, tag="qr",
                                      name=f"qr_{d}{tb}")
                        for h in range(HPC):
                            hs_ = slice(h * 96, (h + 1) * 96)
                            nc.vector.tensor_tensor(shf[:, hs_], shf[:, hs_],
                                                    b_s, MUL)
                            nc.vector.tensor_tensor(q_sb[:, hs_], q_sb[:, hs_],
                                                    a_s, MUL)
                        nc.vector.tensor_tensor(qr, q_sb, shf, ADD)
                        dst = qT if d == "q" else kT
                        for h in range(HPC):
                            pt = ppt.tile([HD, 128], f32, tag="pt",
                                          name=f"pt_{d}{tb}_{h}")
                            nc.tensor.transpose(
                                pt, qr[:, h * 96:(h + 1) * 96], ident)
                            nc.scalar.copy(dst[(g, h)][:, col:col + 128], pt)
                    # drain one pending attention instance per sub-tile
                    if pending:
                        attn_instance(*pending.pop(0))
                if blk % 4 == 3:
                    pending.extend((g, h, qh)
                                   for h in range(HPC) for qh in range(2))
            while pending:
                attn_instance(*pending.pop(0))
    nc.compile()
    return nc


def _build_launch2():
    import concourse.mybir as mybir
    import concourse.tile as tile
    from concourse import bacc

    f32 = mybir.dt.float32
    bf16 = os.environ.get("KERNEL_L2_BF16", "1") == "1"
    mm = mybir.dt.bfloat16 if bf16 else _mm_dt()
    TOK = 1024           # tokens per core
    NW = 1152            # outdims per core
    MB = NW // 128       # 9 outdim blocks
    nc = bacc.Bacc("TRN2", target_bir_lowering=False, debug=False)

    at_d = nc.dram_tensor("attnT", [DIM, TOK], mm, kind="ExternalInput").ap()
    wo_d = nc.dram_tensor("woj", [DIM, NW], mm, kind="ExternalInput").ap()
    bo_d = nc.dram_tensor("boj", [1, NW], f32, kind="ExternalInput").ap()
    # transposed output [outdim, tok]; host transposes back
    out_d = nc.dram_tensor("out", [NW, TOK], f32, kind="ExternalOutput").ap()

    with tile.TileContext(nc) as tc:
        ats, wos = [], []
        with (
            tc.tile_pool(name="singles2", bufs=1) as singles,
            tc.tile_pool(name="atp", bufs=KC) as atp,
            tc.tile_pool(name="wop", bufs=KC) as wop,
            tc.tile_pool(name="outp", bufs=4) as outp,
            tc.tile_pool(name="psp", bufs=8, space="PSUM") as psp,
        ):
            bo_sb = singles.tile([128, MB], f32, tag="bo_sb", name="bo_sb")
            nc.sync.dma_start(bo_sb,
                              bo_d.rearrange("a (m p) -> p (a m)", p=128))
            for kc in range(KC):
                a = atp.tile([128, TOK], mm, tag="at", name=f"at{kc}")
                nc.sync.dma_start(a, at_d[kc * 128:(kc + 1) * 128, :])
                ats.append(a)
                w = wop.tile([128, NW], mm, tag="wo", name=f"wo{kc}")
                nc.sync.dma_start(w, wo_d[kc * 128:(kc + 1) * 128, :])
                wos.append(w)
            # chunk-outer accumulation over groups of 4 outdim blocks
            # (8 psum banks per group) so the PE tracks the DMA feed instead
            # of serializing behind it.
            units = [(mb, th) for mb in range(MB) for th in range(2)]
            ots = {}
            for base in range(0, len(units), 8):
                grp = units[base:base + 8]
                pss = {}
                for mb, th in grp:
                    pss[(mb, th)] = psp.tile([128, 512], f32, tag="ps",
                                             name=f"ps{mb}_{th}")
                for kc in range(KC):
                    for mb, th in grp:
                        nc.tensor.matmul(
                            pss[(mb, th)], wos[kc][:, mb * 128:(mb + 1) * 128],
                            ats[kc][:, th * 512:(th + 1) * 512],
                            start=(kc == 0), stop=(kc == KC - 1))
                for mb, th in grp:
                    if mb not in ots:
                        ots[mb] = outp.tile([128, TOK], f32, tag="ot",
                                            name=f"ot{mb}")
                    nc.vector.tensor_scalar_add(
                        ots[mb][:, th * 512:(th + 1) * 512], pss[(mb, th)],
                        bo_sb[:, mb:mb + 1])
                    if th == 1:
                        nc.sync.dma_start(out_d[mb * 128:(mb + 1) * 128, :],
                                          ots[mb])
    nc.compile()
    return nc


def _get(name, builder):
    if name not in _CACHE:
        _CACHE[name] = builder()
    return _CACHE[name]


def _rope_tables(frame, height, width):
    t = np.repeat(np.arange(frame), height * width)
    y = np.tile(np.repeat(np.arange(height), width), frame)
    x = np.tile(np.arange(width), frame * height)
    D = HD // 3
    A = np.empty((S, HD), np.float32)
    B = np.empty((S, HD), np.float32)
    for i, pos in enumerate((t, y, x)):
        inv = 1.0 / (10000.0 ** (np.arange(0, D, 2, dtype=np.float32) / D))
        f = pos[:, None].astype(np.float32) * inv[None, :]
        A[:, i * D:i * D + 16] = np.cos(f)
        A[:, i * D + 16:(i + 1) * D] = np.cos(f)
        B[:, i * D:i * D + 16] = -np.sin(f)
        B[:, i * D + 16:(i + 1) * D] = np.sin(f)
    return A, B


def _tile_hT(hT):
    # [2304, 4096] -> [NB, 128, KC*TB]: blk-major, partition-major, then
    # (chunk, token) contiguous per partition
    return np.ascontiguousarray(
        hT.reshape(KC, 128, NB, TB).transpose(2, 1, 0, 3).reshape(
            NB, 128, KC * TB))


def _tile_w(w):
    # [2304, CW] -> [128, KC*CW]
    return np.ascontiguousarray(
        w.reshape(KC, 128, CW).transpose(1, 0, 2).reshape(128, KC * CW))


def _tile_rope(a):
    # [4096, 96] -> [NB, 128, 2*96]
    return np.ascontiguousarray(
        a.reshape(NB, 2, 128, HD).transpose(0, 2, 1, 3).reshape(
            NB, 128, 2 * HD))


def kernel(hidden_states, wq, bq, wk, bk, wv, bv, wo, bo, frame, height, width):
    from concourse import bass_utils

    f, hh, ww = int(frame), int(height), int(width)
    hs = np.asarray(hidden_states, dtype=np.float32)
    assert hs.shape == (1, S, DIM) and f * hh * ww == S
    wq, wk, wv, wo = (np.asarray(a, np.float32) for a in (wq, wk, wv, wo))
    bq, bk, bv, bo = (np.asarray(a, np.float32) for a in (bq, bk, bv, bo))

    perm = np.concatenate([np.arange(k, S, SPN) for k in range(SPN)])
    A, B = _rope_tables(f, hh, ww)
    A = _tile_rope(A[perm])
    B = _tile_rope(B[perm])
    hT = _tile_hT(hs[0].T[:, perm])

    nc1 = _get("l1", _build_launch1)
    in1 = []
    for c in range(8):
        sl = slice(c * CW, (c + 1) * CW)
        in1.append({
            "hT": hT,
            "wq": _tile_w(wq[:, sl]),
            "wk": _tile_w(wk[:, sl]),
            "wv": _tile_w(wv[:, sl]),
            "bq": np.ascontiguousarray(bq[sl]).reshape(1, CW),
            "bk": np.ascontiguousarray(bk[sl]).reshape(1, CW),
            "bv": np.ascontiguousarray(bv[sl]).reshape(1, CW),
            "A": A, "B": B,
        })
    res1 = bass_utils.run_bass_kernel_spmd(nc1, in1, core_ids=list(range(8)))
    LAST_RESULTS.append(res1)

    outN = np.concatenate([res1.results[c]["outN"] for c in range(8)], 0)
    attnT_g = (outN[:, :HD, :] / outN[:, HD:HD + 1, :]).reshape(DIM, S)
    attnT = np.empty_like(attnT_g)
    attnT[:, perm] = attnT_g

    nc2 = _get("l2", _build_launch2)
    if os.environ.get("KERNEL_L2_BF16", "1") == "1":
        import ml_dtypes
        l2dt = ml_dtypes.bfloat16
    else:
        l2dt = np.float32
    in2 = []
    for c in range(8):
        i, j = divmod(c, 2)
        in2.append({
            "attnT": np.ascontiguousarray(
                attnT[:, i * 1024:(i + 1) * 1024].astype(l2dt)),
            "woj": np.ascontiguousarray(
                wo[:, j * 1152:(j + 1) * 1152].astype(l2dt)),
            "boj": np.ascontiguousarray(bo[j * 1152:(j + 1) * 1152]).reshape(1, 1152),
        })
    res2 = bass_utils.run_bass_kernel_spmd(nc2, in2, core_ids=list(range(8)))
    LAST_RESULTS.append(res2)

    out = np.empty((S, DIM), np.float32)
    for c in range(8):
        i, j = divmod(c, 2)
        out[i * 1024:(i + 1) * 1024, j * 1152:(j + 1) * 1152] = \
            res2.results[c]["out"].T
    return out[None]
